# revision 11
# baseline (speedup 1.0000x reference)
"""CoxPH loss with Efron ties on 8 trn2 NeuronCores.

Math: with per-time tables over t in [0, 2048):
    s[t] = sum_{d_i=t} exp(hr_i)
    T[t] = sum_{d_i=t, e_i=1} exp(hr_i)
    n[t] = #{d_i=t, e_i=1}
    R[t] = suffix_sum(s)[t]
the Efron correction is
    corr = sum_t sum_{k=0}^{n_t-1} log(R_t - (k/n_t) T_t)
and loss = -(sum hr*e - corr) / (sum e + 1e-7).

Device plan (SPMD on 8 cores):
  Sharding: the loss is permutation-invariant over samples, so the host
  assigns samples to cores so every core receives exactly the same number
  of event samples, laid out events-first (column-major: device column c
  holds samples [128c, 128c+128)).  Per-column sample composition is then
  known at build time: pure-event columns, <=2 boundary (mixed) chunks,
  pure-censored columns.
  phase 1 (histogram via radix one-hots over t = dhi*64 + dlo):
    event columns:    stat = [w*OHhi | OHhi] (64), mov = OHlo (64)
                      -> psum quadrants T (=their s contribution) and n.
    censored columns: stat = [w*OHhi] (32), mov = OHlo (64) -> psum s.
    mixed chunk(s):   baseline scheme stat=[w*OHhi|OHhi] (64),
                      mov=[OHlo|OHlo_e] (128) -> s/T/n quadrants.
  This cuts the one-hot build from 192 to 128 VectorE elems/sample and
  nearly halves the ScalarE broadcast-expansion work.  Prep (digits, exp,
  bf16 pair-packing) is interleaved per 512-column section so it pipelines
  under the chunk loop instead of serializing in front of it.
  AllReduce of the 3x2048 tables across the 8 cores.
  phase 2: R via triangular-ones matmul suffix sum; each core selects its
  own 2 time-columns via a host-provided mask, then runs a masked
  [128,1280] log grid per column with fused Ln+accumulate on ScalarE.
  Output per core: [128, 3] partials (corr, hr*e, n-sum); host does the
  final tiny reduction (the unshard step).
"""

import sys

sys.path.insert(0, "/opt/trn_rl_repo")

import numpy as np

import concourse.bacc as bacc
import concourse.bass as bass
import concourse.mybir as mybir
import concourse.tile as tile

NCORES = 8
N = 4_194_304
NPC = N // NCORES            # 524288 samples per core
P = 128
CTOT = NPC // P              # 4096 free-dim columns of samples
CS = 64                      # chunk size (columns per chunk)
NCHUNK = CTOT // CS          # 64
SEC = 512                    # section size (columns) for interleaved prep
NSEC = CTOT // SEC           # 8
HI = 32                      # top 5 bits of t (d >> 6)
LO = 64                      # low 6 bits of t (d & 63)
NT = 2048                    # t = dhi*64 + dlo
FT = NT // P                 # 16 columns of 128 times
KMAX = 1280                  # static bound on max ties per time
COLS_PER_CORE = FT // NCORES  # 2

F32 = mybir.dt.float32
BF16 = mybir.dt.bfloat16
U16 = mybir.dt.uint16
I32 = mybir.dt.int32
AL = mybir.AluOpType
AF = mybir.ActivationFunctionType

_COMPILED = {}


def build(mc0, mc1, e_end, c_start):
    """mc0..mc1: chunk indices of the mixed region; e_end/c_start: sample
    boundaries (same on every core by construction)."""
    nc = bacc.Bacc("TRN2", target_bir_lowering=False, debug=False, num_devices=NCORES)

    hr_d = nc.dram_tensor("hr", [NPC], F32, kind="ExternalInput")
    dur_d = nc.dram_tensor("dur", [NPC], I32, kind="ExternalInput")
    evt_d = nc.dram_tensor("evt", [NPC], I32, kind="ExternalInput")
    iota32x_d = nc.dram_tensor("iota32x", [P, CS * HI], BF16, kind="ExternalInput")
    iota64x_d = nc.dram_tensor("iota64x", [P, CS * LO], BF16, kind="ExternalInput")
    iota64p1x_d = nc.dram_tensor("iota64p1x", [P, CS * LO], BF16, kind="ExternalInput")
    iotak_d = nc.dram_tensor("iotak", [P, KMAX], F32, kind="ExternalInput")
    tri128_d = nc.dram_tensor("tri128", [P, P], F32, kind="ExternalInput")  # [k,m]=k>=m
    tri16_d = nc.dram_tensor("tri16", [FT, FT], F32, kind="ExternalInput")  # [k,m]=k>m
    ones16_d = nc.dram_tensor("ones16", [FT, P], F32, kind="ExternalInput")
    colsel_d = nc.dram_tensor(
        "colsel", [P, COLS_PER_CORE * FT], F32, kind="ExternalInput"
    )
    out_d = nc.dram_tensor("out", [P, 3], F32, kind="ExternalOutput")

    hr2 = hr_d.ap().rearrange("(p c) -> p c", p=P)
    dur2 = dur_d.ap().rearrange("(p c) -> p c", p=P)
    evt2 = evt_d.ap().rearrange("(p c) -> p c", p=P)

    # section classification for the hr*e partial sums
    # sec covers samples [65536*s, 65536*(s+1))
    sec_kind = []
    for s in range(NSEC):
        lo_s, hi_s = SEC * P * s, SEC * P * (s + 1)
        if hi_s <= e_end:
            sec_kind.append("event")       # all events: sum hr
        elif lo_s >= c_start:
            sec_kind.append("cen")         # all censored: contributes 0
        else:
            sec_kind.append("mixed")       # needs evt data
    # chunk types
    chunk_kind = []
    for ch in range(NCHUNK):
        if ch < mc0:
            chunk_kind.append("event")
        elif ch < mc1:
            chunk_kind.append("mixed")
        else:
            chunk_kind.append("cen")
    n_e_cols = mc0 * CS
    n_m_cols = (mc1 - mc0) * CS
    n_c_cols = (NCHUNK - mc1) * CS

    with tile.TileContext(nc) as tc:
        with (
            tc.tile_pool(name="const", bufs=1) as constp,
            tc.tile_pool(name="acc", bufs=1) as accp,
            tc.tile_pool(name="ps", bufs=1, space="PSUM") as psp,
            tc.tile_pool(name="dram", bufs=1, space="DRAM") as dramp,
        ):
            # ---- constants: dense repeated iota tiles, DMA'd from host ----
            iota32_x = constp.tile([P, CS, HI], BF16)
            nc.sync.dma_start(iota32_x[:], iota32x_d[:].rearrange("p (c j) -> p c j", j=HI))
            iota64_x = constp.tile([P, CS, LO], BF16)
            nc.sync.dma_start(iota64_x[:], iota64x_d[:].rearrange("p (c j) -> p c j", j=LO))
            iota64p1_x = constp.tile([P, CS, LO], BF16)
            nc.sync.dma_start(
                iota64p1_x[:], iota64p1x_d[:].rearrange("p (c j) -> p c j", j=LO)
            )

            hre_secs = accp.tile([P, NSEC], F32)
            nc.vector.memset(hre_secs[:], 0.0)

            # PSUM accumulators (parity pairs relax same-bank accum chains)
            ps_e = psp.tile([LO, LO], F32)      # rows [w*OHhi|OHhi], cols OHlo
            ps_e2 = psp.tile([LO, LO], F32)
            ps_c = psp.tile([HI, LO], F32)      # rows w*OHhi, cols OHlo
            ps_c2 = psp.tile([HI, LO], F32)
            ps_m = psp.tile([LO, P], F32)       # rows [w*OHhi|OHhi], cols [OHlo|OHlo_e]

            g_e = g_c = g_m = 0  # per-stream matmul counters

            with (
                tc.tile_pool(name="sec", bufs=2) as secp,
                tc.tile_pool(name="xp", bufs=2) as xpp,
                tc.tile_pool(name="oh", bufs=2) as ohp,
                tc.tile_pool(name="mix", bufs=1) as mixp,
            ):
                def emit_prep(s):
                    csl = slice(s * SEC, (s + 1) * SEC)
                    dur_sb = secp.tile([P, SEC], I32, tag="dur")
                    hr_sb = secp.tile([P, SEC], F32, tag="hr")
                    nc.sync.dma_start(dur_sb[:], dur2[:, csl])
                    nc.sync.dma_start(hr_sb[:], hr2[:, csl])

                    dlo_i = secp.tile([P, SEC], I32, tag="di")
                    dhi_i = secp.tile([P, SEC], I32, tag="di2")
                    dlo_b = secp.tile([P, SEC], BF16, tag="dlo_b")
                    dhi_b = secp.tile([P, SEC], BF16, tag="dhi_b")
                    w_b = secp.tile([P, SEC], BF16, tag="w_b")
                    nc.vector.tensor_scalar(dlo_i[:], dur_sb[:], 63, None, AL.bitwise_and)
                    nc.vector.tensor_copy(dlo_b[:], dlo_i[:])
                    nc.vector.tensor_scalar(
                        dhi_i[:], dur_sb[:], 6, None, AL.logical_shift_right
                    )
                    nc.vector.tensor_copy(dhi_b[:], dhi_i[:])
                    nc.scalar.activation(w_b[:], hr_sb[:], AF.Exp)

                    # hr*e partial for this section
                    e_b = None
                    if sec_kind[s] == "event":
                        nc.vector.tensor_reduce(
                            hre_secs[:, s : s + 1], hr_sb[:], mybir.AxisListType.X, AL.add
                        )
                    elif sec_kind[s] == "mixed":
                        evt_sb = secp.tile([P, SEC], I32, tag="evt")
                        nc.sync.dma_start(evt_sb[:], evt2[:, csl])
                        e_b = secp.tile([P, SEC], BF16, tag="e_b")
                        nc.vector.tensor_copy(e_b[:], evt_sb[:])
                        scrap_f = secp.tile([P, SEC], F32, tag="scrap_f")
                        nc.vector.scalar_tensor_tensor(
                            scrap_f[:], hr_sb[:], 1.0, e_b[:],
                            AL.mult, AL.mult,
                            accum_out=hre_secs[:, s : s + 1],
                        )

                    # pair-pack bf16 streams: pk = (bits << 16) | bits
                    packs = [("dlo", dlo_b), ("dhi", dhi_b), ("w", w_b)]
                    if sec_kind[s] == "mixed":
                        # dlo1e = (dlo + 1) * e: 1..64 for events, 0 otherwise
                        dlo_e_b = secp.tile([P, SEC], BF16, tag="dlo_e_b")
                        nc.vector.scalar_tensor_tensor(
                            dlo_e_b[:], dlo_b[:], 1.0, e_b[:], AL.add, AL.mult
                        )
                        packs.append(("dlo_e", dlo_e_b))
                    pk = {}
                    for nm, srcb in packs:
                        t32 = secp.tile([P, SEC], I32, tag="di")
                        nc.vector.tensor_copy(t32[:], srcb[:].bitcast(U16))
                        s32 = secp.tile([P, SEC], I32, tag="di2")
                        nc.vector.tensor_scalar(
                            s32[:], t32[:], 16, None, AL.logical_shift_left
                        )
                        pk_t = secp.tile([P, SEC], I32, tag=f"pk_{nm}")
                        nc.vector.tensor_tensor(pk_t[:], s32[:], t32[:], AL.bitwise_or)
                        pk[nm] = pk_t
                    return pk

                def emit_chunks(s, pk):
                    nonlocal g_e, g_c, g_m
                    BC = 2 * CS  # expansion block: 2 chunks of columns
                    for b in range(SEC // BC):
                        bsl = slice(b * BC, (b + 1) * BC)
                        # pair-packed expansions on ScalarE, one op per stream
                        # per 4-chunk block
                        dlo_x = xpp.tile([P, BC, LO // 2], F32, tag="dlo_x")
                        nc.scalar.copy(
                            dlo_x[:],
                            pk["dlo"][:, bsl].bitcast(F32).unsqueeze(2)
                            .broadcast_to([P, BC, LO // 2]),
                        )
                        dhi_x = xpp.tile([P, BC, HI // 2], F32, tag="dhi_x")
                        nc.scalar.copy(
                            dhi_x[:],
                            pk["dhi"][:, bsl].bitcast(F32).unsqueeze(2)
                            .broadcast_to([P, BC, HI // 2]),
                        )
                        w_x = xpp.tile([P, BC, HI // 2], F32, tag="w_x")
                        nc.scalar.copy(
                            w_x[:],
                            pk["w"][:, bsl].bitcast(F32).unsqueeze(2)
                            .broadcast_to([P, BC, HI // 2]),
                        )
                        for cb in range(BC // CS):
                            ch = s * (SEC // CS) + b * (BC // CS) + cb
                            kind = chunk_kind[ch]
                            sl = slice(cb * CS, (cb + 1) * CS)
                            xsl = slice((b * (BC // CS) + cb) * CS,
                                        (b * (BC // CS) + cb + 1) * CS)
                            emit_one_chunk(
                                kind, pk,
                                dlo_x[:, sl, :], dhi_x[:, sl, :], w_x[:, sl, :],
                                xsl,
                            )

                def emit_one_chunk(kind, pk, dlo_xs, dhi_xs, w_xs, xsl):
                    nonlocal g_e, g_c, g_m
                    if kind == "mixed":
                        lhs = mixp.tile([P, CS, P], BF16, tag="lhs_m")
                        dlo_e_x = mixp.tile([P, CS, LO // 2], F32, tag="dlo_e_x")
                        nc.scalar.copy(
                            dlo_e_x[:],
                            pk["dlo_e"][:, xsl].bitcast(F32).unsqueeze(2)
                            .broadcast_to([P, CS, LO // 2]),
                        )
                        nc.vector.tensor_tensor(
                            lhs[:, :, LO : 2 * LO], dlo_e_x[:].bitcast(BF16),
                            iota64p1_x[:], AL.is_equal,
                        )
                    else:
                        lhs = ohp.tile([P, CS, LO], BF16, tag="lhs")
                    nc.vector.tensor_tensor(
                        lhs[:, :, 0:LO], dlo_xs.bitcast(BF16), iota64_x[:],
                        AL.is_equal,
                    )

                    rhs = ohp.tile([P, CS, LO], BF16, tag="rhs")
                    nc.vector.tensor_tensor(
                        rhs[:, :, HI : 2 * HI], dhi_xs.bitcast(BF16),
                        iota32_x[:], AL.is_equal,
                    )
                    nc.vector.tensor_tensor(
                        rhs[:, :, 0:HI], rhs[:, :, HI : 2 * HI],
                        w_xs.bitcast(BF16), AL.mult,
                    )
                    if kind == "event":
                        for c in range(CS):
                            nc.tensor.matmul(
                                ps_e[:] if g_e % 2 == 0 else ps_e2[:],
                                rhs[:, c, :],
                                lhs[:, c, 0:LO],
                                start=(g_e < 2),
                                stop=(g_e >= n_e_cols - 2),
                            )
                            g_e += 1
                    elif kind == "cen":
                        for c in range(CS):
                            nc.tensor.matmul(
                                ps_c[:] if g_c % 2 == 0 else ps_c2[:],
                                rhs[:, c, 0:HI],
                                lhs[:, c, 0:LO],
                                start=(g_c < 2),
                                stop=(g_c >= n_c_cols - 2),
                            )
                            g_c += 1
                    else:
                        for c in range(CS):
                            nc.tensor.matmul(
                                ps_m[:],
                                rhs[:, c, :],
                                lhs[:, c, :],
                                start=(g_m == 0),
                                stop=(g_m == n_m_cols - 1),
                            )
                            g_m += 1

                # software pipeline: emit prep for section s+1 before the
                # chunk work of section s so ScalarE gets a section of
                # lookahead on the VectorE queue
                pk_next = emit_prep(0)
                for s in range(NSEC):
                    pk_cur = pk_next
                    if s + 1 < NSEC:
                        pk_next = emit_prep(s + 1)
                    emit_chunks(s, pk_cur)

            hre_acc = accp.tile([P, 1], F32)
            nc.vector.tensor_reduce(
                hre_acc[:], hre_secs[:], mybir.AxisListType.X, AL.add
            )

            # ---- merge psums into the [96, 64] table (rows: s | T | n) ----
            if n_e_cols == 0:
                nc.vector.memset(ps_e[:], 0.0)
                nc.vector.memset(ps_e2[:], 0.0)
            if n_c_cols == 0:
                nc.vector.memset(ps_c[:], 0.0)
                nc.vector.memset(ps_c2[:], 0.0)
            # m_Tn rows 0:32 = T (partitions 0..31), rows 32:64 = n (32..63)
            m_Tn = accp.tile([LO, LO], F32)
            nc.vector.tensor_copy(m_Tn[:], ps_e[:])
            nc.vector.tensor_tensor(m_Tn[:], m_Tn[:], ps_e2[:], AL.add)
            if n_m_cols > 0:
                nc.vector.tensor_tensor(m_Tn[:], m_Tn[:], ps_m[:, LO:P], AL.add)
            # m_s (partitions 0..31) = ps_c + ps_c2 + T_evt (+ mix s-quadrant)
            m_s = accp.tile([HI, LO], F32)
            nc.vector.tensor_copy(m_s[:], ps_c[:])
            nc.vector.tensor_tensor(m_s[:], m_s[:], ps_c2[:], AL.add)
            nc.vector.tensor_tensor(m_s[:], m_s[:], ps_e[0:HI, :], AL.add)
            nc.vector.tensor_tensor(m_s[:], m_s[:], ps_e2[0:HI, :], AL.add)
            if n_m_cols > 0:
                nc.vector.tensor_tensor(m_s[:], m_s[:], ps_m[0:HI, 0:LO], AL.add)

            ar_in = dramp.tile([3 * NT], F32)
            ar_out = dramp.tile([3 * NT], F32)
            ar_v = ar_in[:].rearrange("(a b) -> a b", a=3 * HI)
            nc.sync.dma_start(ar_v[0:HI, :], m_s[:])
            nc.sync.dma_start(ar_v[HI : 3 * HI, :], m_Tn[:])
            nc.gpsimd.collective_compute(
                "AllReduce",
                AL.add,
                replica_groups=[list(range(NCORES))],
                ins=[ar_in[:].opt()],
                outs=[ar_out[:].opt()],
            )

            # ---- phase 2 ----
            gridp2_cm = tc.tile_pool(name="grid2", bufs=1)
            gridp2 = gridp2_cm.__enter__()
            tri128 = constp.tile([P, P], F32)
            nc.sync.dma_start(tri128[:], tri128_d[:])
            tri16 = constp.tile([FT, FT], F32)
            nc.sync.dma_start(tri16[:], tri16_d[:])
            iotak = constp.tile([P, KMAX], F32)
            nc.sync.dma_start(iotak[:], iotak_d[:])
            ones16 = constp.tile([FT, P], F32)
            nc.sync.dma_start(ones16[:], ones16_d[:])
            colsel = constp.tile([P, COLS_PER_CORE * FT], F32)
            nc.sync.dma_start(colsel[:], colsel_d[:])

            # t = f*128 + p layouts
            s_a = accp.tile([P, FT], F32)
            nc.sync.dma_start(s_a[:], ar_out[0:NT].rearrange("(f p) -> p f", p=P))
            T_a = accp.tile([P, FT], F32)
            nc.sync.dma_start(T_a[:], ar_out[NT : 2 * NT].rearrange("(f p) -> p f", p=P))
            n_a = accp.tile([P, FT], F32)
            nc.sync.dma_start(
                n_a[:], ar_out[2 * NT : 3 * NT].rearrange("(f p) -> p f", p=P)
            )
            s_b = accp.tile([FT, P], F32)  # natural row-major [f, p] view
            nc.sync.dma_start(s_b[:], ar_out[0:NT].rearrange("(f p) -> f p", p=P))

            # R suffix sum: within-column suffix (tri128 @ s_a) plus the
            # cross-column offsets, both accumulated into one PSUM tile
            cs16 = accp.tile([FT, 1], F32)
            nc.vector.tensor_reduce(cs16[:], s_b[:], mybir.AxisListType.X, AL.add)
            csu = accp.tile([FT, FT], F32)
            nc.vector.tensor_scalar(csu[:], tri16[:], cs16[:, 0:1], None, AL.mult)
            rp_ps = psp.tile([P, FT], F32)
            nc.tensor.matmul(rp_ps[:], tri128[:], s_a[:], start=True, stop=False)
            nc.tensor.matmul(rp_ps[:], ones16[:], csu[:], start=False, stop=True)
            R = accp.tile([P, FT], F32)
            nc.vector.tensor_copy(R[:], rp_ps[:])

            n_r = n_a
            n_s = accp.tile([P, FT], F32)
            nc.vector.tensor_scalar_max(n_s[:], n_r[:], 1.0)
            rec = accp.tile([P, FT], F32)
            nc.vector.reciprocal(rec[:], n_s[:])
            Tn = accp.tile([P, FT], F32)
            nc.vector.tensor_tensor(Tn[:], T_a[:], rec[:], AL.mult)
            negTn = accp.tile([P, FT], F32)
            nc.vector.tensor_scalar_mul(negTn[:], Tn[:], -1.0)

            nsum = accp.tile([P, 1], F32)
            nc.vector.tensor_reduce(nsum[:], n_r[:], mybir.AxisListType.X, AL.add)

            corr_cols = accp.tile([P, COLS_PER_CORE], F32)
            for j in range(COLS_PER_CORE):
                msl = slice(j * FT, (j + 1) * FT)
                my_negTn = accp.tile([P, 1], F32, tag="my_negTn")
                mscr = accp.tile([P, FT], F32, tag="mscr")
                nc.vector.tensor_tensor(mscr[:], negTn[:], colsel[:, msl], AL.mult)
                nc.vector.tensor_reduce(my_negTn[:], mscr[:], mybir.AxisListType.X, AL.add)
                my_R = accp.tile([P, 1], F32, tag="my_R")
                nc.vector.tensor_tensor(mscr[:], R[:], colsel[:, msl], AL.mult)
                nc.vector.tensor_reduce(my_R[:], mscr[:], mybir.AxisListType.X, AL.add)
                my_n = accp.tile([P, 1], F32, tag="my_n")
                nc.vector.tensor_tensor(mscr[:], n_r[:], colsel[:, msl], AL.mult)
                nc.vector.tensor_reduce(my_n[:], mscr[:], mybir.AxisListType.X, AL.add)

                arg = gridp2.tile([P, KMAX], F32, tag="arg")
                nc.vector.tensor_scalar(
                    arg[:], iotak[:], my_negTn[:, 0:1], my_R[:, 0:1], AL.mult, AL.add
                )
                mask = gridp2.tile([P, KMAX], F32, tag="mask")
                nc.vector.tensor_scalar(
                    mask[:], iotak[:], my_n[:, 0:1], None, AL.is_lt
                )
                margs = gridp2.tile([P, KMAX], F32, tag="margs")
                nc.vector.scalar_tensor_tensor(
                    margs[:], arg[:], 1.0, mask[:], AL.subtract, AL.mult
                )
                lscrap = gridp2.tile([P, KMAX], F32, tag="lscrap")
                nc.scalar.activation(
                    lscrap[:], margs[:], AF.Ln, bias=1.0,
                    accum_out=corr_cols[:, j : j + 1],
                )
            corr_acc = accp.tile([P, 1], F32)
            nc.vector.tensor_reduce(
                corr_acc[:], corr_cols[:], mybir.AxisListType.X, AL.add
            )

            # ---- output [128, 3] ----
            out_sb = accp.tile([P, 3], F32)
            nc.vector.tensor_copy(out_sb[:, 0:1], corr_acc[:])
            nc.vector.tensor_copy(out_sb[:, 1:2], hre_acc[:])
            nc.vector.tensor_copy(out_sb[:, 2:3], nsum[:])
            nc.sync.dma_start(out_d[:], out_sb[:])
            gridp2_cm.__exit__(None, None, None)

    nc.compile()
    return nc


def _consts():
    iota32 = np.tile(np.arange(HI), (P, 1)).astype(np.float32)
    iota64 = np.tile(np.arange(LO), (P, 1)).astype(np.float32)
    iotak = np.tile(np.arange(KMAX, dtype=np.float32), (P, 1))
    k = np.arange(P)
    tri128 = (k[:, None] >= k[None, :]).astype(np.float32)
    kf = np.arange(FT)
    tri16 = (kf[:, None] > kf[None, :]).astype(np.float32)
    return iota32, iota64, iotak, tri128, tri16


def _plan(evt_flat):
    """Equal-event sharding: per-core stream = [E events][p pool][C censored],
    identical E/p/C on every core.  Returns (perm[NCORES, NPC], mc0, mc1,
    e_end, c_start)."""
    ev_idx = np.flatnonzero(evt_flat)
    cen_idx = np.flatnonzero(evt_flat == 0)
    Etot, Ctot = ev_idx.size, cen_idx.size
    E, C = Etot // NCORES, Ctot // NCORES
    pool = np.concatenate([ev_idx[NCORES * E :], cen_idx[NCORES * C :]])
    p = pool.size // NCORES
    assert NCORES * E + NCORES * C + pool.size == N
    perms = []
    for c in range(NCORES):
        stream = np.concatenate(
            [
                ev_idx[c * E : (c + 1) * E],
                pool[c * p : (c + 1) * p],
                cen_idx[c * C : (c + 1) * C],
            ]
        )
        # column-major: device column k holds samples [128k, 128(k+1))
        perms.append(np.ascontiguousarray(stream.reshape(CTOT, P).T).reshape(-1))
    e_end = E
    c_start = E + p
    mc0 = e_end // (CS * P)
    mc1 = -(-c_start // (CS * P))  # ceil
    mc1 = max(mc1, mc0)
    return perms, mc0, mc1, e_end, c_start


def _in_maps(hazard_ratio, durations, events):
    import ml_dtypes

    hr = np.ascontiguousarray(np.asarray(hazard_ratio, dtype=np.float32).reshape(-1))
    dur = np.ascontiguousarray(np.asarray(durations, dtype=np.int32).reshape(-1))
    evt = np.ascontiguousarray(np.asarray(events, dtype=np.int32).reshape(-1))
    perms, mc0, mc1, e_end, c_start = _plan(evt)

    iota32, iota64, iotak, tri128, tri16 = _consts()
    iota32x = np.tile(np.arange(HI), (P, CS)).astype(ml_dtypes.bfloat16)
    iota64x = np.tile(np.arange(LO), (P, CS)).astype(ml_dtypes.bfloat16)
    iota64p1x = np.tile(np.arange(1, LO + 1), (P, CS)).astype(ml_dtypes.bfloat16)
    ones16 = np.ones((FT, P), dtype=np.float32)

    in_maps = []
    for c in range(NCORES):
        pi = perms[c]
        colsel = np.zeros((P, COLS_PER_CORE * FT), dtype=np.float32)
        for j in range(COLS_PER_CORE):
            colsel[:, j * FT + (c * COLS_PER_CORE + j)] = 1.0
        in_maps.append(
            {
                "hr": hr[pi],
                "dur": dur[pi],
                "evt": evt[pi],
                "iota32x": iota32x,
                "iota64x": iota64x,
                "iota64p1x": iota64p1x,
                "iotak": iotak,
                "tri128": tri128,
                "tri16": tri16,
                "ones16": ones16,
                "colsel": colsel,
            }
        )
    return in_maps, mc0, mc1, e_end, c_start


def _run(hazard_ratio, durations, events, trace=False, tmpdir=None):
    from concourse.bass_utils import run_bass_kernel_spmd

    in_maps, mc0, mc1, e_end, c_start = _in_maps(hazard_ratio, durations, events)
    key = (mc0, mc1, e_end, c_start)
    if key not in _COMPILED:
        _COMPILED.clear()
        _COMPILED[key] = build(*key)
    nc = _COMPILED[key]

    kw = {}
    if trace:
        kw = dict(trace=True, tmpdir=tmpdir)
    res = run_bass_kernel_spmd(nc, in_maps, list(range(NCORES)), **kw)

    outs = [res.results[c]["out"] for c in range(NCORES)]
    corr = np.float32(sum(o[:, 0].sum(dtype=np.float32) for o in outs))
    hre = np.float32(sum(o[:, 1].sum(dtype=np.float32) for o in outs))
    esum = outs[0][:, 2].sum(dtype=np.float32)
    loss = -(hre - corr) / (esum + np.float32(1e-7))
    return np.float32(loss).reshape(()), res


def kernel(hazard_ratio, durations, events):
    out, _ = _run(hazard_ratio, durations, events)
    return out


# revision 13
# speedup vs baseline: 1.0075x; 1.0075x over previous
"""CoxPH loss with Efron ties on 8 trn2 NeuronCores.

Math: with per-time tables over t in [0, 2048):
    s[t] = sum_{d_i=t} exp(hr_i)
    T[t] = sum_{d_i=t, e_i=1} exp(hr_i)
    n[t] = #{d_i=t, e_i=1}
    R[t] = suffix_sum(s)[t]
the Efron correction is
    corr = sum_t sum_{k=0}^{n_t-1} log(R_t - (k/n_t) T_t)
and loss = -(sum hr*e - corr) / (sum e + 1e-7).

Device plan (SPMD on 8 cores):
  Sharding: the loss is permutation-invariant over samples, so the host
  assigns samples to cores so every core receives exactly the same number
  of event samples, laid out events-first (column-major: device column c
  holds samples [128c, 128c+128)).  Per-column sample composition is then
  known at build time: pure-event columns, <=2 boundary (mixed) chunks,
  pure-censored columns.
  phase 1 (histogram via radix one-hots over t = dhi*64 + dlo):
    event columns:    stat = [w*OHhi | OHhi] (64), mov = OHlo (64)
                      -> psum quadrants T (=their s contribution) and n.
    censored columns: stat = [w*OHhi] (32), mov = OHlo (64) -> psum s.
    mixed chunk(s):   baseline scheme stat=[w*OHhi|OHhi] (64),
                      mov=[OHlo|OHlo_e] (128) -> s/T/n quadrants.
  This cuts the one-hot build from 192 to 128 VectorE elems/sample and
  nearly halves the ScalarE broadcast-expansion work.  Prep (digits, exp,
  bf16 pair-packing) is interleaved per 512-column section so it pipelines
  under the chunk loop instead of serializing in front of it.
  AllReduce of the 3x2048 tables across the 8 cores.
  phase 2: R via triangular-ones matmul suffix sum; each core selects its
  own 2 time-columns via a host-provided mask, then runs a masked
  [128,1280] log grid per column with fused Ln+accumulate on ScalarE.
  Output per core: [128, 3] partials (corr, hr*e, n-sum); host does the
  final tiny reduction (the unshard step).
"""

import sys

sys.path.insert(0, "/opt/trn_rl_repo")

import numpy as np

import concourse.bacc as bacc
import concourse.bass as bass
import concourse.mybir as mybir
import concourse.tile as tile

NCORES = 8
N = 4_194_304
NPC = N // NCORES            # 524288 samples per core
P = 128
CTOT = NPC // P              # 4096 free-dim columns of samples
CS = 64                      # chunk size (columns per chunk)
NCHUNK = CTOT // CS          # 64
SEC = 512                    # section size (columns) for interleaved prep
NSEC = CTOT // SEC           # 8
HI = 32                      # top 5 bits of t (d >> 6)
LO = 64                      # low 6 bits of t (d & 63)
NT = 2048                    # t = dhi*64 + dlo
FT = NT // P                 # 16 columns of 128 times
KMAX = 1280                  # static bound on max ties per time
COLS_PER_CORE = FT // NCORES  # 2

F32 = mybir.dt.float32
BF16 = mybir.dt.bfloat16
U16 = mybir.dt.uint16
I32 = mybir.dt.int32
AL = mybir.AluOpType
AF = mybir.ActivationFunctionType

_COMPILED = {}


def build(mc0, mc1, e_end, c_start):
    """mc0..mc1: chunk indices of the mixed region; e_end/c_start: sample
    boundaries (same on every core by construction)."""
    nc = bacc.Bacc("TRN2", target_bir_lowering=False, debug=False, num_devices=NCORES)

    hr_d = nc.dram_tensor("hr", [NPC], F32, kind="ExternalInput")
    dur_d = nc.dram_tensor("dur", [NPC], I32, kind="ExternalInput")
    evt_d = nc.dram_tensor("evt", [NPC], I32, kind="ExternalInput")
    iota32x_d = nc.dram_tensor("iota32x", [P, CS * HI], BF16, kind="ExternalInput")
    iota64x_d = nc.dram_tensor("iota64x", [P, CS * LO], BF16, kind="ExternalInput")
    iota64p1x_d = nc.dram_tensor("iota64p1x", [P, CS * LO], BF16, kind="ExternalInput")
    iotak_d = nc.dram_tensor("iotak", [P, KMAX], F32, kind="ExternalInput")
    tri128_d = nc.dram_tensor("tri128", [P, P], F32, kind="ExternalInput")  # [k,m]=k>=m
    tri16_d = nc.dram_tensor("tri16", [FT, FT], F32, kind="ExternalInput")  # [k,m]=k>m
    ones16_d = nc.dram_tensor("ones16", [FT, P], F32, kind="ExternalInput")
    colsel_d = nc.dram_tensor(
        "colsel", [P, COLS_PER_CORE * FT], F32, kind="ExternalInput"
    )
    out_d = nc.dram_tensor("out", [P, 3], F32, kind="ExternalOutput")

    hr2 = hr_d.ap().rearrange("(p c) -> p c", p=P)
    dur2 = dur_d.ap().rearrange("(p c) -> p c", p=P)
    evt2 = evt_d.ap().rearrange("(p c) -> p c", p=P)

    # section classification for the hr*e partial sums
    # sec covers samples [65536*s, 65536*(s+1))
    sec_kind = []
    for s in range(NSEC):
        lo_s, hi_s = SEC * P * s, SEC * P * (s + 1)
        if hi_s <= e_end:
            sec_kind.append("event")       # all events: sum hr
        elif lo_s >= c_start:
            sec_kind.append("cen")         # all censored: contributes 0
        else:
            sec_kind.append("mixed")       # needs evt data
    # chunk types
    chunk_kind = []
    for ch in range(NCHUNK):
        if ch < mc0:
            chunk_kind.append("event")
        elif ch < mc1:
            chunk_kind.append("mixed")
        else:
            chunk_kind.append("cen")
    n_e_cols = mc0 * CS
    n_m_cols = (mc1 - mc0) * CS
    n_c_cols = (NCHUNK - mc1) * CS

    with tile.TileContext(nc) as tc:
        with (
            tc.tile_pool(name="const", bufs=1) as constp,
            tc.tile_pool(name="acc", bufs=1) as accp,
            tc.tile_pool(name="ps", bufs=1, space="PSUM") as psp,
            tc.tile_pool(name="dram", bufs=1, space="DRAM") as dramp,
        ):
            # ---- constants: dense repeated iota tiles, DMA'd from host ----
            iota32_x = constp.tile([P, CS, HI], BF16)
            nc.sync.dma_start(iota32_x[:], iota32x_d[:].rearrange("p (c j) -> p c j", j=HI))
            iota64_x = constp.tile([P, CS, LO], BF16)
            nc.sync.dma_start(iota64_x[:], iota64x_d[:].rearrange("p (c j) -> p c j", j=LO))
            iota64p1_x = constp.tile([P, CS, LO], BF16)
            nc.sync.dma_start(
                iota64p1_x[:], iota64p1x_d[:].rearrange("p (c j) -> p c j", j=LO)
            )

            hre_secs = accp.tile([P, NSEC], F32)
            nc.vector.memset(hre_secs[:], 0.0)

            # PSUM accumulators (parity pairs relax same-bank accum chains)
            ps_e = psp.tile([LO, LO], F32)      # rows [w*OHhi|OHhi], cols OHlo
            ps_e2 = psp.tile([LO, LO], F32)
            ps_c = psp.tile([HI, LO], F32)      # rows w*OHhi, cols OHlo
            ps_c2 = psp.tile([HI, LO], F32)
            ps_m = psp.tile([LO, P], F32)       # rows [w*OHhi|OHhi], cols [OHlo|OHlo_e]

            g_e = g_c = g_m = 0  # per-stream matmul counters

            with (
                tc.tile_pool(name="sec", bufs=2) as secp,
                tc.tile_pool(name="oh", bufs=2) as ohp,
                tc.tile_pool(name="mix", bufs=1) as mixp,
            ):
                def emit_prep(s):
                    csl = slice(s * SEC, (s + 1) * SEC)
                    dur_sb = secp.tile([P, SEC], I32, tag="dur")
                    hr_sb = secp.tile([P, SEC], F32, tag="hr")
                    nc.sync.dma_start(dur_sb[:], dur2[:, csl])
                    nc.sync.dma_start(hr_sb[:], hr2[:, csl])

                    dlo_i = secp.tile([P, SEC], I32, tag="di")
                    dhi_i = secp.tile([P, SEC], I32, tag="di2")
                    dlo_b = secp.tile([P, SEC], BF16, tag="dlo_b")
                    dhi_b = secp.tile([P, SEC], BF16, tag="dhi_b")
                    w_b = secp.tile([P, SEC], BF16, tag="w_b")
                    nc.vector.tensor_scalar(dlo_i[:], dur_sb[:], 63, None, AL.bitwise_and)
                    nc.vector.tensor_copy(dlo_b[:], dlo_i[:])
                    nc.vector.tensor_scalar(
                        dhi_i[:], dur_sb[:], 6, None, AL.logical_shift_right
                    )
                    nc.vector.tensor_copy(dhi_b[:], dhi_i[:])
                    nc.scalar.activation(w_b[:], hr_sb[:], AF.Exp)

                    # hr*e partial for this section
                    e_b = None
                    if sec_kind[s] == "event":
                        nc.vector.tensor_reduce(
                            hre_secs[:, s : s + 1], hr_sb[:], mybir.AxisListType.X, AL.add
                        )
                    elif sec_kind[s] == "mixed":
                        evt_sb = secp.tile([P, SEC], I32, tag="evt")
                        nc.sync.dma_start(evt_sb[:], evt2[:, csl])
                        e_b = secp.tile([P, SEC], BF16, tag="e_b")
                        nc.vector.tensor_copy(e_b[:], evt_sb[:])
                        scrap_f = secp.tile([P, SEC], F32, tag="scrap_f")
                        nc.vector.scalar_tensor_tensor(
                            scrap_f[:], hr_sb[:], 1.0, e_b[:],
                            AL.mult, AL.mult,
                            accum_out=hre_secs[:, s : s + 1],
                        )

                    # pair-pack bf16 streams: pk = (bits << 16) | bits
                    packs = [("dlo", dlo_b), ("dhi", dhi_b), ("w", w_b)]
                    if sec_kind[s] == "mixed":
                        # dlo1e = (dlo + 1) * e: 1..64 for events, 0 otherwise
                        dlo_e_b = secp.tile([P, SEC], BF16, tag="dlo_e_b")
                        nc.vector.scalar_tensor_tensor(
                            dlo_e_b[:], dlo_b[:], 1.0, e_b[:], AL.add, AL.mult
                        )
                        packs.append(("dlo_e", dlo_e_b))
                    pk = {}
                    for nm, srcb in packs:
                        t32 = secp.tile([P, SEC], I32, tag="di")
                        nc.vector.tensor_copy(t32[:], srcb[:].bitcast(U16))
                        s32 = secp.tile([P, SEC], I32, tag="di2")
                        nc.vector.tensor_scalar(
                            s32[:], t32[:], 16, None, AL.logical_shift_left
                        )
                        pk_t = secp.tile([P, SEC], I32, tag=f"pk_{nm}")
                        nc.vector.tensor_tensor(pk_t[:], s32[:], t32[:], AL.bitwise_or)
                        pk[nm] = pk_t
                    return pk

                def pbc(pk_t, xsl, w):
                    # packed [b|b] i32 stream -> broadcast AP [P, CS, w, 2]
                    # whose innermost step-1 bf16 pair keeps DVE 2x mode
                    v = pk_t[:].bitcast(BF16)[:, 2 * xsl.start : 2 * xsl.stop]
                    v = v.rearrange("p (c two) -> p c two", two=2).unsqueeze(2)
                    return v.broadcast_to([P, CS, w, 2])

                def j2(ap):
                    # [P, CS, J] -> [P, CS, J/2, 2]
                    return ap.rearrange("p c (j t) -> p c j t", t=2)

                def emit_chunks(s, pk):
                    for c8 in range(SEC // CS):
                        ch = s * (SEC // CS) + c8
                        emit_one_chunk(
                            chunk_kind[ch], pk, slice(c8 * CS, (c8 + 1) * CS)
                        )

                def emit_one_chunk(kind, pk, xsl):
                    nonlocal g_e, g_c, g_m
                    if kind == "mixed":
                        lhs = mixp.tile([P, CS, P], BF16, tag="lhs_m")
                        nc.vector.tensor_tensor(
                            j2(lhs[:, :, LO : 2 * LO]), pbc(pk["dlo_e"], xsl, LO // 2),
                            j2(iota64p1_x[:]), AL.is_equal,
                        )
                    else:
                        lhs = ohp.tile([P, CS, LO], BF16, tag="lhs")
                    nc.vector.tensor_tensor(
                        j2(lhs[:, :, 0:LO]), pbc(pk["dlo"], xsl, LO // 2),
                        j2(iota64_x[:]), AL.is_equal,
                    )

                    rhs = ohp.tile([P, CS, LO], BF16, tag="rhs")
                    nc.vector.tensor_tensor(
                        j2(rhs[:, :, HI : 2 * HI]), pbc(pk["dhi"], xsl, HI // 2),
                        j2(iota32_x[:]), AL.is_equal,
                    )
                    nc.vector.tensor_tensor(
                        j2(rhs[:, :, 0:HI]), j2(rhs[:, :, HI : 2 * HI]),
                        pbc(pk["w"], xsl, HI // 2), AL.mult,
                    )
                    if kind == "event":
                        for c in range(CS):
                            nc.tensor.matmul(
                                ps_e[:] if g_e % 2 == 0 else ps_e2[:],
                                rhs[:, c, :],
                                lhs[:, c, 0:LO],
                                start=(g_e < 2),
                                stop=(g_e >= n_e_cols - 2),
                            )
                            g_e += 1
                    elif kind == "cen":
                        for c in range(CS):
                            nc.tensor.matmul(
                                ps_c[:] if g_c % 2 == 0 else ps_c2[:],
                                rhs[:, c, 0:HI],
                                lhs[:, c, 0:LO],
                                start=(g_c < 2),
                                stop=(g_c >= n_c_cols - 2),
                            )
                            g_c += 1
                    else:
                        for c in range(CS):
                            nc.tensor.matmul(
                                ps_m[:],
                                rhs[:, c, :],
                                lhs[:, c, :],
                                start=(g_m == 0),
                                stop=(g_m == n_m_cols - 1),
                            )
                            g_m += 1

                # software pipeline: emit prep for section s+1 before the
                # chunk work of section s so ScalarE gets a section of
                # lookahead on the VectorE queue
                pk_next = emit_prep(0)
                for s in range(NSEC):
                    pk_cur = pk_next
                    if s + 1 < NSEC:
                        pk_next = emit_prep(s + 1)
                    emit_chunks(s, pk_cur)

            hre_acc = accp.tile([P, 1], F32)
            nc.vector.tensor_reduce(
                hre_acc[:], hre_secs[:], mybir.AxisListType.X, AL.add
            )

            # ---- merge psums into the [96, 64] table (rows: s | T | n) ----
            if n_e_cols == 0:
                nc.vector.memset(ps_e[:], 0.0)
                nc.vector.memset(ps_e2[:], 0.0)
            if n_c_cols == 0:
                nc.vector.memset(ps_c[:], 0.0)
                nc.vector.memset(ps_c2[:], 0.0)
            # m_Tn rows 0:32 = T (partitions 0..31), rows 32:64 = n (32..63)
            m_Tn = accp.tile([LO, LO], F32)
            nc.vector.tensor_copy(m_Tn[:], ps_e[:])
            nc.vector.tensor_tensor(m_Tn[:], m_Tn[:], ps_e2[:], AL.add)
            if n_m_cols > 0:
                nc.vector.tensor_tensor(m_Tn[:], m_Tn[:], ps_m[:, LO:P], AL.add)
            # m_s (partitions 0..31) = ps_c + ps_c2 + T_evt (+ mix s-quadrant)
            m_s = accp.tile([HI, LO], F32)
            nc.vector.tensor_copy(m_s[:], ps_c[:])
            nc.vector.tensor_tensor(m_s[:], m_s[:], ps_c2[:], AL.add)
            nc.vector.tensor_tensor(m_s[:], m_s[:], ps_e[0:HI, :], AL.add)
            nc.vector.tensor_tensor(m_s[:], m_s[:], ps_e2[0:HI, :], AL.add)
            if n_m_cols > 0:
                nc.vector.tensor_tensor(m_s[:], m_s[:], ps_m[0:HI, 0:LO], AL.add)

            ar_in = dramp.tile([3 * NT], F32)
            ar_out = dramp.tile([3 * NT], F32)
            ar_v = ar_in[:].rearrange("(a b) -> a b", a=3 * HI)
            nc.sync.dma_start(ar_v[0:HI, :], m_s[:])
            nc.sync.dma_start(ar_v[HI : 3 * HI, :], m_Tn[:])
            nc.gpsimd.collective_compute(
                "AllReduce",
                AL.add,
                replica_groups=[list(range(NCORES))],
                ins=[ar_in[:].opt()],
                outs=[ar_out[:].opt()],
            )

            # ---- phase 2 ----
            gridp2_cm = tc.tile_pool(name="grid2", bufs=1)
            gridp2 = gridp2_cm.__enter__()
            tri128 = constp.tile([P, P], F32)
            nc.sync.dma_start(tri128[:], tri128_d[:])
            tri16 = constp.tile([FT, FT], F32)
            nc.sync.dma_start(tri16[:], tri16_d[:])
            iotak = constp.tile([P, KMAX], F32)
            nc.sync.dma_start(iotak[:], iotak_d[:])
            ones16 = constp.tile([FT, P], F32)
            nc.sync.dma_start(ones16[:], ones16_d[:])
            colsel = constp.tile([P, COLS_PER_CORE * FT], F32)
            nc.sync.dma_start(colsel[:], colsel_d[:])

            # t = f*128 + p layouts
            s_a = accp.tile([P, FT], F32)
            nc.sync.dma_start(s_a[:], ar_out[0:NT].rearrange("(f p) -> p f", p=P))
            T_a = accp.tile([P, FT], F32)
            nc.sync.dma_start(T_a[:], ar_out[NT : 2 * NT].rearrange("(f p) -> p f", p=P))
            n_a = accp.tile([P, FT], F32)
            nc.sync.dma_start(
                n_a[:], ar_out[2 * NT : 3 * NT].rearrange("(f p) -> p f", p=P)
            )
            s_b = accp.tile([FT, P], F32)  # natural row-major [f, p] view
            nc.sync.dma_start(s_b[:], ar_out[0:NT].rearrange("(f p) -> f p", p=P))

            # R suffix sum: within-column suffix (tri128 @ s_a) plus the
            # cross-column offsets, both accumulated into one PSUM tile
            cs16 = accp.tile([FT, 1], F32)
            nc.vector.tensor_reduce(cs16[:], s_b[:], mybir.AxisListType.X, AL.add)
            csu = accp.tile([FT, FT], F32)
            nc.vector.tensor_scalar(csu[:], tri16[:], cs16[:, 0:1], None, AL.mult)
            rp_ps = psp.tile([P, FT], F32)
            nc.tensor.matmul(rp_ps[:], tri128[:], s_a[:], start=True, stop=False)
            nc.tensor.matmul(rp_ps[:], ones16[:], csu[:], start=False, stop=True)
            R = accp.tile([P, FT], F32)
            nc.vector.tensor_copy(R[:], rp_ps[:])

            n_r = n_a
            n_s = accp.tile([P, FT], F32)
            nc.vector.tensor_scalar_max(n_s[:], n_r[:], 1.0)
            rec = accp.tile([P, FT], F32)
            nc.vector.reciprocal(rec[:], n_s[:])
            Tn = accp.tile([P, FT], F32)
            nc.vector.tensor_tensor(Tn[:], T_a[:], rec[:], AL.mult)
            negTn = accp.tile([P, FT], F32)
            nc.vector.tensor_scalar_mul(negTn[:], Tn[:], -1.0)

            nsum = accp.tile([P, 1], F32)
            nc.vector.tensor_reduce(nsum[:], n_r[:], mybir.AxisListType.X, AL.add)

            corr_cols = accp.tile([P, COLS_PER_CORE], F32)
            for j in range(COLS_PER_CORE):
                msl = slice(j * FT, (j + 1) * FT)
                my_negTn = accp.tile([P, 1], F32, tag="my_negTn")
                mscr = accp.tile([P, FT], F32, tag="mscr")
                nc.vector.tensor_tensor(mscr[:], negTn[:], colsel[:, msl], AL.mult)
                nc.vector.tensor_reduce(my_negTn[:], mscr[:], mybir.AxisListType.X, AL.add)
                my_R = accp.tile([P, 1], F32, tag="my_R")
                nc.vector.tensor_tensor(mscr[:], R[:], colsel[:, msl], AL.mult)
                nc.vector.tensor_reduce(my_R[:], mscr[:], mybir.AxisListType.X, AL.add)
                my_n = accp.tile([P, 1], F32, tag="my_n")
                nc.vector.tensor_tensor(mscr[:], n_r[:], colsel[:, msl], AL.mult)
                nc.vector.tensor_reduce(my_n[:], mscr[:], mybir.AxisListType.X, AL.add)

                arg = gridp2.tile([P, KMAX], F32, tag="arg")
                nc.vector.tensor_scalar(
                    arg[:], iotak[:], my_negTn[:, 0:1], my_R[:, 0:1], AL.mult, AL.add
                )
                mask = gridp2.tile([P, KMAX], F32, tag="mask")
                nc.vector.tensor_scalar(
                    mask[:], iotak[:], my_n[:, 0:1], None, AL.is_lt
                )
                margs = gridp2.tile([P, KMAX], F32, tag="margs")
                nc.vector.scalar_tensor_tensor(
                    margs[:], arg[:], 1.0, mask[:], AL.subtract, AL.mult
                )
                lscrap = gridp2.tile([P, KMAX], F32, tag="lscrap")
                nc.scalar.activation(
                    lscrap[:], margs[:], AF.Ln, bias=1.0,
                    accum_out=corr_cols[:, j : j + 1],
                )
            corr_acc = accp.tile([P, 1], F32)
            nc.vector.tensor_reduce(
                corr_acc[:], corr_cols[:], mybir.AxisListType.X, AL.add
            )

            # ---- output [128, 3] ----
            out_sb = accp.tile([P, 3], F32)
            nc.vector.tensor_copy(out_sb[:, 0:1], corr_acc[:])
            nc.vector.tensor_copy(out_sb[:, 1:2], hre_acc[:])
            nc.vector.tensor_copy(out_sb[:, 2:3], nsum[:])
            nc.sync.dma_start(out_d[:], out_sb[:])
            gridp2_cm.__exit__(None, None, None)

    nc.compile()
    return nc


def _consts():
    iota32 = np.tile(np.arange(HI), (P, 1)).astype(np.float32)
    iota64 = np.tile(np.arange(LO), (P, 1)).astype(np.float32)
    iotak = np.tile(np.arange(KMAX, dtype=np.float32), (P, 1))
    k = np.arange(P)
    tri128 = (k[:, None] >= k[None, :]).astype(np.float32)
    kf = np.arange(FT)
    tri16 = (kf[:, None] > kf[None, :]).astype(np.float32)
    return iota32, iota64, iotak, tri128, tri16


def _plan(evt_flat):
    """Equal-event sharding: per-core stream = [E events][p pool][C censored],
    identical E/p/C on every core.  Returns (perm[NCORES, NPC], mc0, mc1,
    e_end, c_start)."""
    ev_idx = np.flatnonzero(evt_flat)
    cen_idx = np.flatnonzero(evt_flat == 0)
    Etot, Ctot = ev_idx.size, cen_idx.size
    E, C = Etot // NCORES, Ctot // NCORES
    pool = np.concatenate([ev_idx[NCORES * E :], cen_idx[NCORES * C :]])
    p = pool.size // NCORES
    assert NCORES * E + NCORES * C + pool.size == N
    perms = []
    for c in range(NCORES):
        stream = np.concatenate(
            [
                ev_idx[c * E : (c + 1) * E],
                pool[c * p : (c + 1) * p],
                cen_idx[c * C : (c + 1) * C],
            ]
        )
        # column-major: device column k holds samples [128k, 128(k+1))
        perms.append(np.ascontiguousarray(stream.reshape(CTOT, P).T).reshape(-1))
    e_end = E
    c_start = E + p
    mc0 = e_end // (CS * P)
    mc1 = -(-c_start // (CS * P))  # ceil
    mc1 = max(mc1, mc0)
    return perms, mc0, mc1, e_end, c_start


def _in_maps(hazard_ratio, durations, events):
    import ml_dtypes

    hr = np.ascontiguousarray(np.asarray(hazard_ratio, dtype=np.float32).reshape(-1))
    dur = np.ascontiguousarray(np.asarray(durations, dtype=np.int32).reshape(-1))
    evt = np.ascontiguousarray(np.asarray(events, dtype=np.int32).reshape(-1))
    perms, mc0, mc1, e_end, c_start = _plan(evt)

    iota32, iota64, iotak, tri128, tri16 = _consts()
    iota32x = np.tile(np.arange(HI), (P, CS)).astype(ml_dtypes.bfloat16)
    iota64x = np.tile(np.arange(LO), (P, CS)).astype(ml_dtypes.bfloat16)
    iota64p1x = np.tile(np.arange(1, LO + 1), (P, CS)).astype(ml_dtypes.bfloat16)
    ones16 = np.ones((FT, P), dtype=np.float32)

    in_maps = []
    for c in range(NCORES):
        pi = perms[c]
        colsel = np.zeros((P, COLS_PER_CORE * FT), dtype=np.float32)
        for j in range(COLS_PER_CORE):
            colsel[:, j * FT + (c * COLS_PER_CORE + j)] = 1.0
        in_maps.append(
            {
                "hr": hr[pi],
                "dur": dur[pi],
                "evt": evt[pi],
                "iota32x": iota32x,
                "iota64x": iota64x,
                "iota64p1x": iota64p1x,
                "iotak": iotak,
                "tri128": tri128,
                "tri16": tri16,
                "ones16": ones16,
                "colsel": colsel,
            }
        )
    return in_maps, mc0, mc1, e_end, c_start


def _run(hazard_ratio, durations, events, trace=False, tmpdir=None):
    from concourse.bass_utils import run_bass_kernel_spmd

    in_maps, mc0, mc1, e_end, c_start = _in_maps(hazard_ratio, durations, events)
    key = (mc0, mc1, e_end, c_start)
    if key not in _COMPILED:
        _COMPILED.clear()
        _COMPILED[key] = build(*key)
    nc = _COMPILED[key]

    kw = {}
    if trace:
        kw = dict(trace=True, tmpdir=tmpdir)
    res = run_bass_kernel_spmd(nc, in_maps, list(range(NCORES)), **kw)

    outs = [res.results[c]["out"] for c in range(NCORES)]
    corr = np.float32(sum(o[:, 0].sum(dtype=np.float32) for o in outs))
    hre = np.float32(sum(o[:, 1].sum(dtype=np.float32) for o in outs))
    esum = outs[0][:, 2].sum(dtype=np.float32)
    loss = -(hre - corr) / (esum + np.float32(1e-7))
    return np.float32(loss).reshape(()), res


def kernel(hazard_ratio, durations, events):
    out, _ = _run(hazard_ratio, durations, events)
    return out
